# revision 3
# baseline (speedup 1.0000x reference)
"""Trainium2 Bass kernel for nn_PositionEncoding (embedding lookup + sincos
position encoding + mask select).

Design:
  - bf16 output (rel tol 2e-2; bf16 noise ~2e-3).  ACT writes sin/cos
    directly in bf16; store is 8 MiB/core.
  - Wrap via magic-number round: s = (t + 2^23) - 2^23 (one 2x tensor_scalar
    pass), y = s - t = -wrap(t) in [-0.5, 0.5]; sin = Sin(-2pi*y),
    cos = Sin(pi/2 - 2pi*|y|).
  - No mask on device: the host zeroes the residues of class tokens, so the
    sincos columns evaluate to exactly [0, 1, 0, 1, ...] there; the class
    table has that pattern pre-subtracted from its rows.  The merge is a
    single bf16 tensor_tensor add (2x 16-bit DVE mode) with non-class tokens
    gathering the all-zero row 4096.
  - Class table bf16 padded to 128 cols (256 B descriptors); merge reads
    the first 64 bf16 of each row.  Within each tile the host permutes
    tokens class-first and sorts by class id, so only 4096 positions are
    gathered (vs 8192) and consecutive descriptors read ascending table
    rows (DMA coalescing); pads target ascending zero rows.  Gathers run
    512 indices per chunk across all 4 SWDGE queues, single_packet=True
    (each of these measured: chunk size and queue spread are worth ~5x,
    sorted ids ~1.5x, single_packet another ~1.4x).
  - DVE software pipeline: merge of tile k-1 issues after mult/mod of tile k
    so ACT(k) never waits behind a merge.  e/g/r/i quad-buffered; the SP
    queue only runs stores (loads live on Pool) so no load ever queues
    behind a store's semaphore wait.

Per-core layout: 8 tiles x 8192 tokens; tile token (p, j) = p*64 + j.
The gather's position i lands at partition i%128, block i//128, and reads
index slot (i%16, i//16) of the [16, 512] wrapped idx layout -- the host
permutes class_ids accordingly.
"""
import os
os.environ.setdefault("JAX_PLATFORMS", "axon")
import math
import numpy as np
import ml_dtypes

import concourse.bacc as bacc
import concourse.bass as bass
import concourse.mybir as mybir
from concourse.library_config import mlp

B, S = 64, 8192
L = 32                 # encode levels
E = 64                 # 2*L
EP = 128               # padded table row width (bf16 -> 256B rows)
CLASS_NUM = 4096
NCORES = 8
TPC = B * S // NCORES  # tokens per core = 65536
NTILE = 8
TT = TPC // NTILE      # tokens per tile = 8192
NB = 64                # tokens per partition per tile
NG = 4                 # level groups
GL = 8                 # levels per group

# Class-first gather: within each tile the host permutes tokens so class
# tokens occupy the lowest gather positions; only SGATH positions are
# gathered/merged (n_class ~ Binom(8192, 0.5) = 4096 +- 45; class tokens
# past the bound -- about half the tiles have a few dozen -- are patched
# with exact f32 rows on the host).
NSGU = 8               # gather chunks per tile (two per SWDGE queue)
SCH = 512              # indices per dma_gather
SGATH = NSGU * SCH     # gathered positions per tile (4096)
MBLK = SGATH // 128    # merged blocks per partition (32)

PI32 = np.float32(math.pi)
MAGIC = float(np.float32(2.0 ** 23))

_CACHED_NC = None


def _build_nc():
    nc = bacc.Bacc("TRN2", debug=False, num_swdge_queues=4)
    f32, i16, i64 = mybir.dt.float32, mybir.dt.int16, mybir.dt.int64
    i32 = mybir.dt.int32
    bf16 = mybir.dt.bfloat16
    Alu = mybir.AluOpType

    # bf16 table rows padded to 128 cols (256 B gather descriptors).  The
    # NEFF-visible dtype is int32 (pairs) -- keeps host<->device transfer on
    # plain numpy dtypes; the gather reads a bf16 view.
    tbl32 = nc.dram_tensor("tbl", [CLASS_NUM + SGATH, EP // 2], i32, kind="ExternalInput")
    tbl = tbl32.bitcast(bf16)
    resid = nc.dram_tensor("resid", [NTILE * 128, NG * NB], f32, kind="ExternalInput")
    idx = nc.dram_tensor("idx", [NTILE * 128, SGATH // 16], i16, kind="ExternalInput")
    fcst = nc.dram_tensor("fcst", [128, L], f32, kind="ExternalInput")
    out = nc.dram_tensor("out", [NTILE * 128, NB * E], bf16, kind="ExternalOutput")

    HW = NB * L            # half-width free size (2048): one slot per (j, level)
    FW = NB * E            # full width (4096)
    GW = NB * EP           # gather buffer bf16 width (8192)

    from contextlib import ExitStack
    with ExitStack() as _es:
        def sb(name, shape, dt):
            return _es.enter_context(nc.sbuf_tensor(name, shape, dt))

        def sem(name):
            return _es.enter_context(nc.semaphore(name))

        f_sb = sb("f_sb", [128, L], f32)
        b_cos = sb("b_cos", [128, 1], f32)   # pi/2
        rbuf = [sb(f"r{i}", [128, NG * NB], f32) for i in range(4)]
        ibuf = [sb(f"i{i}", [128, SGATH // 16], i16) for i in range(4)]
        tbuf = [sb(f"t{i}", [128, HW], f32) for i in range(2)]
        sbuf_ = [sb(f"s{i}", [128, HW], f32) for i in range(2)]
        ebuf = [sb(f"e{i}", [128, FW], bf16) for i in range(4)]
        gbuf = [sb(f"g{i}", [128, GW], bf16) for i in range(4)]
        lr = [sem(f"lr{i}") for i in range(4)]   # resid loads: +16 per use
        li = [sem(f"li{i}") for i in range(4)]   # idx loads
        # gathers: one sem per (SWDGE queue, g buffer) pair
        gqs = [[sem(f"gq{q}_{i}") for i in range(4)] for q in range(4)]
        st = [sem(f"st{i}") for i in range(4)]   # stores
        vt = sem("vt")    # DVE t-mult consumed resid: +1 per tile
        vu = sem("vu")    # DVE u (wrap) ready: +1 per tile
        ad = sem("ad")    # ACT passes: +3 per tile
        vp = sem("vp")    # merge done: +1 per tile
        cs = sem("cs")    # consts ready

        with nc.Block() as block:

            @block.sync
            def _(sync):
                sync.dma_start(f_sb[:], fcst[:]).then_inc(cs, 16)
                for k in range(NTILE):
                    # store of tile k once its merge is done
                    sync.wait_ge(vp, k + 1)
                    sync.dma_start(
                        out[k * 128:(k + 1) * 128, :], ebuf[k % 4][:]
                    ).then_inc(st[k % 4], 16)
                for i in range(4):
                    sync.wait_ge(st[i], 16 * (NTILE // 4))

            @block.gpsimd
            def _(gpsimd):
                gpsimd.load_library(mlp)
                gpsimd.memset(b_cos[:], float(PI32 / 2)).then_inc(cs, 1)

                def resid_load(k):
                    if k >= 4:
                        # r[k%4] consumed by t-mult of tile k-4
                        gpsimd.wait_ge(vt, k - 3)
                    gpsimd.dma_start(
                        rbuf[k % 4][:], resid[k * 128:(k + 1) * 128, :]
                    ).then_inc(lr[k % 4], 16)

                for k in range(4):
                    resid_load(k)
                for k in range(NTILE):
                    b = k % 4
                    if k >= 4:
                        # idx buffer released at gather(k-4) DMA completion
                        for q in range(4):
                            gpsimd.wait_ge(gqs[q][b], 16 * (NSGU // 4) * (k // 4))
                    gpsimd.dma_start(
                        ibuf[b][:], idx[k * 128:(k + 1) * 128, :]
                    ).then_inc(li[b], 16)
                    if k >= 4:
                        # g buffer consumed by merge of tile k-4
                        gpsimd.wait_ge(vp, k - 3)
                    gpsimd.wait_ge(li[b], 16 * (k // 4 + 1))
                    for c in range(NSGU):
                        gpsimd.dma_gather(
                            bass.AP(gbuf[b], c * (SCH // 128) * EP,
                                    [[GW, 128], [EP, SCH // 128], [1, EP]]),
                            tbl[:],
                            bass.AP(ibuf[b], c * (SCH // 16),
                                    [[SGATH // 16, 128], [1, SCH // 16]]),
                            SCH, SCH, EP, single_packet=True,
                            queue_num=c % 4,
                        ).then_inc(gqs[c % 4][b], 16)
                    if k + 4 < NTILE:
                        resid_load(k + 4)

            def merge(vector, k):
                e, g = ebuf[k % 4], gbuf[k % 4]
                vector.wait_ge(ad, 3 * (k + 1))          # cos(k) done
                for q in range(4):
                    vector.wait_ge(gqs[q][k % 4], 16 * (NSGU // 4) * (k // 4 + 1))
                vector.tensor_tensor(
                    bass.AP(e, 0, [[FW, 128], [E, MBLK], [1, E]]),
                    bass.AP(e, 0, [[FW, 128], [E, MBLK], [1, E]]),
                    bass.AP(g, 0, [[GW, 128], [EP, MBLK], [1, E]]),
                    Alu.add,
                ).then_inc(vp, 1)

            @block.vector
            def _(vector):
                vector.wait_ge(cs, 17)
                for k in range(NTILE):
                    b = k % 2
                    vector.wait_ge(lr[k % 4], 16 * (k // 4 + 1))  # resid loaded
                    if k >= 2:
                        # t buffer free once ACT abs(k-2) has read it
                        vector.wait_ge(ad, 3 * (k - 2) + 2)
                    t, s, r = tbuf[b], sbuf_[b], rbuf[k % 4]
                    # t[p, j*32 + g*8 + l] = F[g*8+l] * r[p, g*64 + j]
                    vector.tensor_tensor(
                        bass.AP(t, 0, [[HW, 128], [L, NB], [GL, NG], [1, GL]]),
                        bass.AP(f_sb, 0, [[L, 128], [0, NB], [GL, NG], [1, GL]]),
                        bass.AP(r, 0, [[NG * NB, 128], [1, NB], [NB, NG], [0, GL]]),
                        Alu.mult,
                    ).then_inc(vt, 1)
                    vector.drain()
                    if k >= 2:
                        # s buffer free once ACT cos(k-2) has read it
                        vector.wait_ge(ad, 3 * (k - 2) + 3)
                    # s = round_even(t) via (t + 2^23) - 2^23
                    vector.tensor_scalar(
                        s[:], t[:], MAGIC, MAGIC, Alu.add, Alu.subtract)
                    vector.drain()
                    # y = s - t = -wrap(t) in [-0.5, 0.5], in place over t
                    vector.tensor_tensor(
                        t[:], s[:], t[:], Alu.subtract,
                    ).then_inc(vu, 1)
                    # software pipeline: merge of the previous tile
                    if k >= 1:
                        merge(vector, k - 1)
                merge(vector, NTILE - 1)

            @block.scalar
            def _(scalar):
                scalar.wait_ge(cs, 17)
                for k in range(NTILE):
                    b = k % 2
                    t, s, e = tbuf[b], sbuf_[b], ebuf[k % 4]
                    scalar.wait_ge(vu, k + 1)
                    if k >= 4:
                        scalar.wait_ge(st[k % 4], 16 * (k // 4))  # e stored
                    # even cols: sin = Sin(-2pi*y)
                    scalar.activation(
                        bass.AP(e, 0, [[FW, 128], [E, NB], [2, L]]),
                        t[:].rearrange("p (j l) -> p j l", l=L),
                        mybir.ActivationFunctionType.Sin,
                        bias=0.0, scale=float(-2.0 * PI32),
                    ).then_inc(ad, 1)
                    # s = |y|  (round values in s no longer needed)
                    scalar.activation(
                        s[:], t[:], mybir.ActivationFunctionType.Abs,
                        bias=0.0, scale=1.0,
                    ).then_inc(ad, 1)
                    # sem (not drain): enforce Abs writeback before the read
                    scalar.wait_ge(ad, 3 * k + 2)
                    # odd cols: cos = Sin(-2pi*|y| + pi/2)
                    scalar.activation(
                        bass.AP(e, 1, [[FW, 128], [E, NB], [2, L]]),
                        s[:].rearrange("p (j l) -> p j l", l=L),
                        mybir.ActivationFunctionType.Sin,
                        bias=b_cos[:, 0:1], scale=float(-2.0 * PI32),
                    ).then_inc(ad, 1)

    nc.compile()
    return nc


def _host_prep(values, E_class, class_ids, is_class):
    """Split across cores and build device-layout input arrays.

    Returns (in_maps, perms, repairs): perms[c][tile*8192 + slot] is the
    core-local token stored at SBUF slot (tile, slot); repairs is a list of
    (core, token, class_id) rows the device did not gather (class count
    above SGATH in some tile) to be patched on the host.
    """
    v = np.ascontiguousarray(values, dtype=np.float32).reshape(-1)
    ids = np.ascontiguousarray(class_ids, dtype=np.int32).reshape(-1)
    m = np.ascontiguousarray(is_class, dtype=np.int32).reshape(-1)

    w = (v * PI32).astype(np.float32)
    q = w.astype(np.float64) / np.float64(math.pi)
    # group residues, float64 -> f32; zeroed at class tokens so the sincos
    # columns there become exactly [0, 1, 0, 1, ...]
    notc = (m == 0)
    resid_full = np.empty((NG, v.size), np.float32)
    for g in range(NG):
        resid_full[g] = np.where(
            notc, np.mod(q * (2.0 ** (g * GL - 1)), 1.0), 0.0
        ).astype(np.float32)

    # gather position i <-> SBUF slot s:  s(i) = (i%128)*64 + i//128,
    # i(s) = (s%64)*128 + s//64
    s_arr = np.arange(TT, dtype=np.int64)
    i_of_slot = (s_arr % NB) * 128 + s_arr // NB     # [8192]

    # bf16 table, rows padded to 128 cols, [0,1,0,1...] pre-subtracted, plus
    # the all-zero row 4096; NEFF-visible dtype int32
    # bf16 table, [0,1,0,1...] pre-subtracted; rows CLASS_NUM.. are zero --
    # ascending pad targets so padded gather descriptors stay coalescible
    tbl_pad = np.zeros((CLASS_NUM + SGATH, EP), np.float32)
    tbl_pad[:CLASS_NUM, :E] = np.asarray(E_class, dtype=np.float32)
    tbl_pad[:CLASS_NUM, 1:E:2] -= 1.0
    tbl_pad = np.ascontiguousarray(
        tbl_pad.astype(ml_dtypes.bfloat16)).view(np.int32)

    fcst = np.broadcast_to(
        (np.float32(2.0) ** (np.arange(L, dtype=np.float32) % GL)), (128, L)
    ).copy()

    in_maps, perms, repairs = [], [], []
    for c in range(NCORES):
        sl = slice(c * TPC, (c + 1) * TPC)
        rc = resid_full[:, sl]                        # [4, 65536]
        idc = ids[sl].reshape(NTILE, TT)
        mc = m[sl].reshape(NTILE, TT)

        r_dev = np.empty((NTILE, 128, NG * NB), np.float32)
        idx_dev = np.empty((NTILE, 128, SGATH // 16), np.int16)
        perm = np.empty(TPC, np.int64)
        for kt in range(NTILE):
            mt, it = mc[kt], idc[kt]
            # class tokens first, ordered by class id so the gather sweeps
            # the table nearly sequentially (DRAM page locality)
            token_at_pos = np.lexsort((it, mt == 0))
            token_at_slot = token_at_pos[i_of_slot]   # [8192] core-tile-local
            perm[kt * TT:(kt + 1) * TT] = kt * TT + token_at_slot

            # resid by slot: [g, slot] -> [p, g*64 + j]
            r_slot = rc[:, kt * TT + token_at_slot]   # [4, 8192]
            r_dev[kt] = (r_slot.reshape(NG, 128, NB)
                         .transpose(1, 0, 2).reshape(128, NG * NB))

            # idx for gathered positions only
            tp = token_at_pos[:SGATH]
            vals = np.where(mt[tp] != 0, it[tp], 0)
            pad = mt[tp] == 0   # tail positions (sorted class-first)
            vals[pad] = CLASS_NUM + np.arange(int(pad.sum()))
            idx_dev[kt] = np.tile(
                vals.reshape(SGATH // 16, 16).T, (8, 1)).astype(np.int16)

            # class tokens beyond the gather bound: repaired on host
            over = token_at_pos[SGATH:]
            over = over[mt[over] != 0]
            for t in over:
                repairs.append((c, kt * TT + int(t), int(it[t])))

        in_maps.append({
            "tbl": tbl_pad,
            "resid": np.ascontiguousarray(r_dev.reshape(NTILE * 128, NG * NB)),
            "idx": np.ascontiguousarray(idx_dev.reshape(NTILE * 128, SGATH // 16)),
            "fcst": fcst,
        })
        perms.append(perm)
    return in_maps, perms, repairs


def kernel(values, E_class, class_ids, is_class):
    global _CACHED_NC
    if _CACHED_NC is None:
        _CACHED_NC = _build_nc()
    nc = _CACHED_NC

    in_maps, perms, repairs = _host_prep(values, E_class, class_ids, is_class)

    from concourse.bass_utils import run_bass_kernel_spmd
    res = run_bass_kernel_spmd(nc, in_maps, core_ids=list(range(NCORES)))

    E_f32 = np.asarray(E_class, dtype=np.float32)
    outs = []
    for c in range(NCORES):
        o = np.asarray(res.results[c]["out"]).astype(np.float32).reshape(TPC, E)
        oc = np.empty((TPC, E), np.float32)
        oc[perms[c]] = o                              # slot -> token order
        outs.append(oc)
    for c, t, cid in repairs:
        outs[c][t] = E_f32[cid]
    full = np.concatenate(outs, axis=0)               # [524288, 64]
    return full.reshape(B, S, E)
